# revision 1
# baseline (speedup 1.0000x reference)
"""Multi-head attention Trainium2 kernel (8 NeuronCores).

Problem: B=2, T=2048, E=1024, H=16, D=64 multi-head attention
    q/k/v = einsum('bte,hed->bhtd', x, W{q,k,v})
    out   = softmax(q k^T / sqrt(D)) v, heads concat, @ Wo, + x

Sharding: data-parallel over batch (2 groups of 4 cores) x tensor-parallel
over heads (4 heads per core). Each core computes, for its batch b and its
4 heads, the partial output  partial = concat_heads(attn) @ Wo[head rows].
The host sums the 4 partials per batch and adds the residual x.

Device layout notes:
  - All matmul inputs are bf16 (fp32 PSUM accumulation); exp runs on the
    scalar engine in fp32 reading scores straight from PSUM.
  - Scores are computed transposed, S^T[T', t], so that P^T = exp(S^T)
    lands directly in the layout the PV matmul needs as its moving
    operand (contraction over T' on partitions).
  - The softmax denominator is obtained by appending a ones-column to V:
    lhsT = [V_h | 1] gives PSUM rows 0..63 = (P V)^T and row 64 = sum(P).
  - Normalization multiplies by a DMA-broadcast reciprocal row (the
    compute engines cannot broadcast along partitions; DMA can).
  - exp is not max-subtracted: scores/8 lie in roughly [-10, 10] for this
    problem family, far inside fp32 exp range.
"""

import contextlib
import ctypes
import os
import sys
import types

import numpy as np
import ml_dtypes

B, T, E, H = 2, 2048, 1024, 16
D = E // H          # 64
NCORES = 8
DP = 2              # batch groups
TPC = NCORES // DP  # cores per batch group
HLOC = H // TPC     # heads per core = 4
CLOC = HLOC * D     # local concat width = 256

_cached_nc = None
LAST_EXEC_NS = None


def _ensure_ntff_hook():
    """bass_utils' trace path imports antenv.axon_hooks, which is absent in
    this image. Recreate it (registry + ctypes NTFF driver) so profiled runs
    don't crash; no-op if the module already exists."""
    try:
        import antenv.axon_hooks  # noqa: F401
        return
    except ImportError:
        pass
    try:
        import antenv
    except ImportError:
        return

    mod = types.ModuleType("antenv.axon_hooks")
    _state = {"hook": None}
    mod.set_axon_ntff_profile_hook = lambda h: _state.__setitem__("hook", h)
    mod.get_axon_ntff_profile_hook = lambda: _state["hook"]
    sys.modules["antenv.axon_hooks"] = mod
    antenv.axon_hooks = mod

    so_path = "/opt/axon/libaxon_pjrt.so"
    if not os.path.exists(so_path):
        return
    try:
        lib = ctypes.CDLL(so_path)
    except OSError:
        return
    if not hasattr(lib, "axon_start_nrt_profile"):
        return
    lib.axon_start_nrt_profile.argtypes = [
        ctypes.POINTER(ctypes.c_int64),
        ctypes.c_size_t,
    ]
    lib.axon_start_nrt_profile.restype = ctypes.c_int64
    lib.axon_stop_nrt_profile.argtypes = [ctypes.c_char_p]
    lib.axon_stop_nrt_profile.restype = ctypes.c_int64

    @contextlib.contextmanager
    def _hook(output_dir, device_ids):
        import jax

        jax.devices()
        if device_ids:
            ids = (ctypes.c_int64 * len(device_ids))(*device_ids)
            rc = lib.axon_start_nrt_profile(ids, len(device_ids))
        else:
            rc = lib.axon_start_nrt_profile(None, 0)
        if rc != 0:
            raise RuntimeError(f"axon_start_nrt_profile rc={rc}")
        try:
            yield
        finally:
            n = lib.axon_stop_nrt_profile(str(output_dir).encode())
            print(f"ntff profile: {n} file(s) -> {output_dir}", file=sys.stderr)

    mod.set_axon_ntff_profile_hook(_hook)


def _build_program():
    import concourse.mybir as mybir
    import concourse.tile as tile
    from concourse import bacc
    from concourse.alu_op_type import AluOpType
    from concourse.tile_rust import add_dep_helper

    def _inst(bi):
        return bi.ins if hasattr(bi, "ins") else bi

    f32 = mybir.dt.float32
    bf16 = mybir.dt.bfloat16
    AF = mybir.ActivationFunctionType

    nc = bacc.Bacc("TRN2", target_bir_lowering=False, debug=False,
                   num_devices=NCORES)

    xT = nc.declare_dram_parameter("xT", [E, T], bf16, isOutput=False)
    wq = nc.declare_dram_parameter("wq", [E, CLOC], bf16, isOutput=False)
    wk = nc.declare_dram_parameter("wk", [E, CLOC], bf16, isOutput=False)
    wv = nc.declare_dram_parameter("wv", [E, CLOC], bf16, isOutput=False)
    wo = nc.declare_dram_parameter("wo", [CLOC, E], bf16, isOutput=False)
    out = nc.declare_dram_parameter("out", [T, E], f32, isOutput=True)

    KC = E // 128        # 8 contraction chunks for the projections
    NT = T // 128        # 16 T'-tiles (key rows per tile)
    NTC = T // 512       # 4 t-chunks (query columns per chunk)
    NPAIR = HLOC // 2    # 2 head pairs

    # PSUM budget (8 banks of 2KB/partition):
    #   bigps: one shared ring of [128,2,512] tiles (scores, projections,
    #          out-proj all share it) -> 3 bufs x 2 banks = 6 banks
    #   pvpsum: PV pair accumulator + denominator bank     = 2 banks
    with tile.TileContext(nc) as tc:
        with (
            tc.tile_pool(name="persist", bufs=1) as persist,
            tc.tile_pool(name="bigps", bufs=3, space="PSUM") as bigps,
            tc.tile_pool(name="pvpsum", bufs=1, space="PSUM") as pvpsum,
            tc.tile_pool(name="ptile", bufs=6) as ptile,
            tc.tile_pool(name="small", bufs=6) as small,
            tc.tile_pool(name="dscratch", bufs=8, space="DRAM") as dscratch,
        ):
            # ---- stage inputs in SBUF (all bf16) ----
            # Many small DMAs spread across the 16 queues; ordered so the
            # first Q/K projection groups unblock as early as possible.
            xT_sb = persist.tile([128, KC, T], bf16)
            wq_sb = persist.tile([128, KC, CLOC], bf16)
            wk_sb = persist.tile([128, KC, CLOC], bf16)
            wv_sb = persist.tile([128, KC, CLOC], bf16)
            # issue the loads from three idle sequencers in parallel; the
            # first K/Q projection groups need wk/wq plane kc and the xT
            # t-halves in kc order
            xT_r = xT.ap().rearrange("(a p) t -> p a t", p=128)
            wq_r = wq.ap().rearrange("(a p) c -> p a c", p=128)
            wk_r = wk.ap().rearrange("(a p) c -> p a c", p=128)
            wv_r = wv.ap().rearrange("(a p) c -> p a c", p=128)
            # sync issues wk + the xT t-half each kc's first matmuls need,
            # interleaved and split by partition halves (same 2KB bursts,
            # more queue parallelism); gpsimd issues the rest; the scalar
            # engine issues nothing so exp starts unimpeded
            for kc in range(KC):
                nc.sync.dma_start(out=wk_sb[:, kc, :], in_=wk_r[:, kc, :])
                nc.sync.dma_start(
                    out=xT_sb[0:64, kc, 0:1024], in_=xT_r[0:64, kc, 0:1024]
                )
                nc.sync.dma_start(
                    out=xT_sb[64:128, kc, 0:1024],
                    in_=xT_r[64:128, kc, 0:1024],
                )
                nc.gpsimd.dma_start(out=wq_sb[:, kc, :], in_=wq_r[:, kc, :])
            for kc in range(KC):
                nc.gpsimd.dma_start(out=wv_sb[:, kc, :], in_=wv_r[:, kc, :])
                nc.sync.dma_start(
                    out=xT_sb[0:64, kc, 1024:T], in_=xT_r[0:64, kc, 1024:T]
                )
                nc.sync.dma_start(
                    out=xT_sb[64:128, kc, 1024:T],
                    in_=xT_r[64:128, kc, 1024:T],
                )
            # Wo rows for head pair pp live at partitions 0..127 of plane pp.
            wo_sb = persist.tile([128, HLOC // 2, E], bf16)
            wo_r = wo.ap().rearrange("(pp r) e -> r pp e", r=128)
            for pp in range(HLOC // 2):
                nc.gpsimd.dma_start(out=wo_sb[:, pp, :], in_=wo_r[:, pp, :])

            # ---- projections ----
            # Q^T, K^T: [CLOC, T] with head-local d on partitions
            # (M-group mg holds heads 2mg, 2mg+1).
            qT_sb = persist.tile([128, NPAIR, T], bf16)
            kT_sb = persist.tile([128, NPAIR, T], bf16)
            # V natural layout per t-tile: [t(128), tile, c]
            vp_sb = persist.tile([128, NT, CLOC], bf16)
            # ones column for the softmax-denominator matmuls
            ones_sb = persist.tile([128, 1], bf16)
            nc.vector.memset(ones_sb[:], 1.0)

            def proj_half(w_sb, dst, mg, tc2):
                """One projection group: head pair mg, t-half tc2 (two
                t-chunks per PSUM tile, one eviction copy per tile)."""
                ps = bigps.tile([128, 2, 512], f32, tag="big")
                for half in range(2):
                    tcn = 2 * tc2 + half
                    for kc in range(KC):
                        nc.tensor.matmul(
                            ps[:, half, :],
                            lhsT=w_sb[:, kc, mg * 128 : (mg + 1) * 128],
                            rhs=xT_sb[:, kc, tcn * 512 : (tcn + 1) * 512],
                            start=(kc == 0),
                            stop=(kc == KC - 1),
                        )
                nc.vector.tensor_copy(
                    out=dst[:, mg, tc2 * 1024 : (tc2 + 1) * 1024],
                    in_=ps[:].rearrange("p a b -> p (a b)"),
                )

            def v_proj_pair(tt0):
                # V tiles tt0, tt0+1 in natural [t, c] layout via x^T as the
                # stationary side; both share one ring slot and one eviction
                # copy so the jit projection doesn't thrash the scores ring
                ps = bigps.tile([128, 2, 512], f32, tag="big")
                for half in range(2):
                    tt = tt0 + half
                    for kc in range(KC):
                        nc.tensor.matmul(
                            ps[:, half, 0:CLOC],
                            lhsT=xT_sb[:, kc, tt * 128 : (tt + 1) * 128],
                            rhs=wv_sb[:, kc, :],
                            start=(kc == 0),
                            stop=(kc == KC - 1),
                        )
                nc.vector.tensor_copy(
                    out=vp_sb[:, tt0 : tt0 + 2, :], in_=ps[:, :, 0:CLOC]
                )

            # headsN[c_lo, pair, t]: plane `pair` holds heads 2p (partitions
            # 0..63) and 2p+1 (64..127) — ready as out-proj stationary tiles.
            headsN = persist.tile([128, NPAIR, T], bf16)

            def attention(pair, tcns, jit_vproj=False):
                h0, h1 = 2 * pair, 2 * pair + 1
                for tcn in tcns:
                    tsl = slice(tcn * 512, (tcn + 1) * 512)
                    # PV pair accumulator (col-packed: h0 -> partitions 0..63,
                    # h1 -> 64..127) and denominator bank (4 accumulators at
                    # partitions 0/32/64/96: h0-even, h1-even, h0-odd, h1-odd
                    # T'-tiles).
                    pv = pvpsum.tile([128, 512], f32, tag="pv")
                    dn = pvpsum.tile([97, 512], f32, tag="dn")
                    # Process T'-tiles in pairs, batching same-shaped matmuls
                    # back-to-back — the PE only pipelines (drain under next
                    # fill) within runs of same-configuration instructions.
                    for g in range(NT // 2):
                        st = (g == 0)
                        sp = (g == NT // 2 - 1)
                        pss, pts = [], []
                        last_s = None
                        for i in range(2):
                            tt = 2 * g + i
                            ksl = slice(tt * 128, (tt + 1) * 128)
                            ps_s = bigps.tile([128, 2, 512], f32, tag="big")
                            pss.append(ps_s)
                            # S^T for both heads (row-packed on the PE)
                            nc.tensor.matmul(
                                ps_s[:, 0, :],
                                lhsT=kT_sb[0:64, pair, ksl],
                                rhs=qT_sb[0:64, pair, tsl],
                                tile_position=(0, 0),
                            )
                            last_s = nc.tensor.matmul(
                                ps_s[:, 1, :],
                                lhsT=kT_sb[64:128, pair, ksl],
                                rhs=qT_sb[64:128, pair, tsl],
                                tile_position=(64, 0),
                            )
                        for i in range(2):
                            pt = ptile.tile([128, 2, 512], bf16, tag="pt")
                            pts.append(pt)
                            nc.scalar.activation(
                                out=pt[:], in_=pss[i][:], func=AF.Exp,
                                scale=0.125,
                            )
                        if jit_vproj and tcn == 0:
                            # first consumer of these V tiles: project them
                            # now so the PE fills exp-wait slack
                            v_proj_pair(2 * g)
                        for i in range(2):
                            tt = 2 * g + i
                            mm = nc.tensor.matmul(
                                pv[0:64, :],
                                lhsT=vp_sb[:, tt, h0 * D : (h0 + 1) * D],
                                rhs=pts[i][:, 0, :],
                                start=(st and i == 0), stop=(sp and i == 1),
                                tile_position=(0, 0),
                            )
                            if i == 0:
                                # keep the two scores groups adjacent on the
                                # PE (same-config runs pipeline; interleaved
                                # configs pay the full isolated matmul cost)
                                add_dep_helper(
                                    _inst(mm), _inst(last_s),
                                    reason="batch scores before pv",
                                )
                            nc.tensor.matmul(
                                pv[64:128, :],
                                lhsT=vp_sb[:, tt, h1 * D : (h1 + 1) * D],
                                rhs=pts[i][:, 1, :],
                                start=(st and i == 0), stop=(sp and i == 1),
                                tile_position=(0, 64),
                            )
                        # denominators: 4 col-tiled M=1 matmuls as one group
                        for par, (i, hh) in (
                            (0, (0, 0)), (32, (0, 1)), (64, (1, 0)),
                            (96, (1, 1)),
                        ):
                            nc.tensor.matmul(
                                dn[par : par + 1, :],
                                lhsT=ones_sb[:],
                                rhs=pts[i][:, hh, :],
                                start=st, stop=sp,
                                tile_position=(0, par),
                            )
                    # evacuate PSUM quickly so the next chunk's PV can start
                    stg = small.tile([128, 512], f32, tag="stg")
                    nc.vector.tensor_copy(out=stg[:], in_=pv[:])
                    dnc = small.tile([97, 512], f32, tag="dnc")
                    nc.vector.tensor_copy(out=dnc[:], in_=dn[:])
                    # denominator = even + odd partial (summed by an
                    # accumulating DMA into the DRAM bounce used for the
                    # partition broadcast), then one reciprocal over the
                    # broadcast tile
                    den = small.tile([128, 512], f32, tag="den")
                    for j, (pe, po) in enumerate(((0, 64), (32, 96))):
                        dsc = dscratch.tile([1, 512], f32, tag="dsc")
                        nc.sync.dma_start(out=dsc[:], in_=dnc[pe : pe + 1, :])
                        nc.gpsimd.dma_start(
                            out=dsc[:],
                            in_=dnc[po : po + 1, :],
                            accum_op=AluOpType.add,
                        )
                        nc.sync.dma_start(
                            out=den[j * 64 : (j + 1) * 64, :],
                            in_=dsc[:].to_broadcast([64, 512]),
                        )
                    rec = small.tile([128, 512], f32, tag="recb")
                    nc.vector.reciprocal_approx_fast(out=rec[:], in_=den[:])
                    nc.vector.tensor_mul(
                        out=headsN[:, pair, tsl], in0=stg[:], in1=rec[:]
                    )

            def out_proj():
                # partial = headsN^T @ Wo_loc ; both e-chunks share a tile
                for tt in range(NT):
                    ksl = slice(tt * 128, (tt + 1) * 128)
                    ps_o = bigps.tile([128, 2, 512], f32, tag="big")
                    for ec in range(2):
                        esl = slice(ec * 512, (ec + 1) * 512)
                        for pp in range(NPAIR):
                            nc.tensor.matmul(
                                ps_o[:, ec, :],
                                lhsT=headsN[:, pp, ksl],
                                rhs=wo_sb[:, pp, esl],
                                start=(pp == 0),
                                stop=(pp == NPAIR - 1),
                            )
                    stg = ptile.tile([128, 2, 512], f32, tag="ostg")
                    nc.vector.tensor_copy(out=stg[:], in_=ps_o[:])
                    nc.sync.dma_start(
                        out=out.ap()[ksl, :],
                        in_=stg[:].rearrange("p a b -> p (a b)"),
                    )

            # Emission order IS program order (Tile tracks deps in trace
            # order) and doubles as scheduler priority. Project only what the
            # next attention chunk needs (scores want all of K^T but just the
            # current t-half of Q^T) so the scalar engine starts exp as early
            # as possible; V tiles are projected just-in-time in pair-0's
            # first t-chunk; out-proj fills pair-1's slack.
            for pair in range(NPAIR):
                proj_half(wk_sb, kT_sb, pair, 0)
                proj_half(wk_sb, kT_sb, pair, 1)
                proj_half(wq_sb, qT_sb, pair, 0)
                attention(pair, (0, 1), jit_vproj=(pair == 0))
                proj_half(wq_sb, qT_sb, pair, 1)
                attention(pair, (2, 3))
            out_proj()

    nc.compile()
    return nc


def _get_program():
    global _cached_nc
    if _cached_nc is None:
        _cached_nc = _build_program()
    return _cached_nc


def kernel(x, Wq, Wk, Wv, Wo):
    global LAST_EXEC_NS
    _ensure_ntff_hook()
    from concourse.bass_utils import run_bass_kernel_spmd

    nc = _get_program()
    bf16 = ml_dtypes.bfloat16

    x = np.asarray(x, dtype=np.float32)
    in_maps = []
    for c in range(NCORES):
        b = c // TPC
        hs = (c % TPC) * HLOC
        xT_c = np.ascontiguousarray(x[b].T).astype(bf16)
        # [HLOC, E, D] -> [E, HLOC*D]
        wq_c = np.ascontiguousarray(
            np.asarray(Wq)[hs : hs + HLOC].transpose(1, 0, 2).reshape(E, CLOC)
        ).astype(bf16)
        wk_c = np.ascontiguousarray(
            np.asarray(Wk)[hs : hs + HLOC].transpose(1, 0, 2).reshape(E, CLOC)
        ).astype(bf16)
        wv_c = np.ascontiguousarray(
            np.asarray(Wv)[hs : hs + HLOC].transpose(1, 0, 2).reshape(E, CLOC)
        ).astype(bf16)
        wo_c = np.ascontiguousarray(
            np.asarray(Wo)[hs * D : (hs + HLOC) * D, :]
        ).astype(bf16)
        in_maps.append(
            {"xT": xT_c, "wq": wq_c, "wk": wk_c, "wv": wv_c, "wo": wo_c}
        )

    trace = bool(os.environ.get("KERNEL_TRACE"))
    res = run_bass_kernel_spmd(nc, in_maps, list(range(NCORES)), trace=trace)
    LAST_EXEC_NS = res.exec_time_ns

    out = np.empty((B, T, E), dtype=np.float32)
    for b in range(B):
        acc = x[b].copy()
        for g in range(TPC):
            acc += res.results[b * TPC + g]["out"]
        out[b] = acc
    return out



# revision 16
# speedup vs baseline: 1.0093x; 1.0093x over previous
"""Multi-head attention Trainium2 kernel (8 NeuronCores).

Problem: B=2, T=2048, E=1024, H=16, D=64 multi-head attention
    q/k/v = einsum('bte,hed->bhtd', x, W{q,k,v})
    out   = softmax(q k^T / sqrt(D)) v, heads concat, @ Wo, + x

Sharding: data-parallel over batch (2 groups of 4 cores) x tensor-parallel
over heads (4 heads per core). Each core computes, for its batch b and its
4 heads, the partial output  partial = concat_heads(attn) @ Wo[head rows].
The host sums the 4 partials per batch and adds the residual x.

Device layout notes:
  - All matmul inputs are bf16 (fp32 PSUM accumulation); exp runs on the
    scalar engine in fp32 reading scores straight from PSUM.
  - Scores are computed transposed, S^T[T', t], so that P^T = exp(S^T)
    lands directly in the layout the PV matmul needs as its moving
    operand (contraction over T' on partitions).
  - The softmax denominator rides along in the PV matmul: each head's V
    tile carries a ones column ([V_h | 1], M=65), so PSUM rows 0..63 of
    the per-head accumulator are (P V)^T and row 64 is sum(P). This
    replaces the v1 M=1 denominator matmuls (256 x ~160ns of pure PE
    column-streaming).
  - Each head accumulates in its own [65,512] PSUM bank (65+65 > 128
    partitions, so the two heads can no longer be column-packed). The
    h1 block is moved to partitions 64..127 of headsN by a small
    SBUF->SBUF DMA (compute engines cannot shift partitions; DMA can).
  - The denominator reciprocal is taken on the [1,512] row, then
    DMA-broadcast along partitions.
  - exp is not max-subtracted: scores/8 lie in roughly [-10, 10] for this
    problem family, far inside fp32 exp range.
  - out-proj tiles are interleaved with pair-1 attention chunks so the
    output DMA streams throughout instead of draining at the end.
"""

import contextlib
import ctypes
import os
import sys
import types

import numpy as np
import ml_dtypes

B, T, E, H = 2, 2048, 1024, 16
D = E // H          # 64
NCORES = 8
DP = 2              # batch groups
TPC = NCORES // DP  # cores per batch group
HLOC = H // TPC     # heads per core = 4
CLOC = HLOC * D     # local concat width = 256

_cached_nc = None
LAST_EXEC_NS = None


def _ensure_ntff_hook():
    """bass_utils' trace path imports antenv.axon_hooks, which is absent in
    this image. Recreate it (registry + ctypes NTFF driver) so profiled runs
    don't crash; no-op if the module already exists."""
    try:
        import antenv.axon_hooks  # noqa: F401
        return
    except ImportError:
        pass
    try:
        import antenv
    except ImportError:
        return

    mod = types.ModuleType("antenv.axon_hooks")
    _state = {"hook": None}
    mod.set_axon_ntff_profile_hook = lambda h: _state.__setitem__("hook", h)
    mod.get_axon_ntff_profile_hook = lambda: _state["hook"]
    sys.modules["antenv.axon_hooks"] = mod
    antenv.axon_hooks = mod

    so_path = "/opt/axon/libaxon_pjrt.so"
    if not os.path.exists(so_path):
        return
    try:
        lib = ctypes.CDLL(so_path)
    except OSError:
        return
    if not hasattr(lib, "axon_start_nrt_profile"):
        return
    lib.axon_start_nrt_profile.argtypes = [
        ctypes.POINTER(ctypes.c_int64),
        ctypes.c_size_t,
    ]
    lib.axon_start_nrt_profile.restype = ctypes.c_int64
    lib.axon_stop_nrt_profile.argtypes = [ctypes.c_char_p]
    lib.axon_stop_nrt_profile.restype = ctypes.c_int64

    @contextlib.contextmanager
    def _hook(output_dir, device_ids):
        import jax

        jax.devices()
        if device_ids:
            ids = (ctypes.c_int64 * len(device_ids))(*device_ids)
            rc = lib.axon_start_nrt_profile(ids, len(device_ids))
        else:
            rc = lib.axon_start_nrt_profile(None, 0)
        if rc != 0:
            raise RuntimeError(f"axon_start_nrt_profile rc={rc}")
        try:
            yield
        finally:
            n = lib.axon_stop_nrt_profile(str(output_dir).encode())
            print(f"ntff profile: {n} file(s) -> {output_dir}", file=sys.stderr)

    mod.set_axon_ntff_profile_hook(_hook)


def _build_program():
    import concourse.mybir as mybir
    import concourse.tile as tile
    from concourse import bacc
    from concourse.tile_rust import add_dep_helper

    def _inst(bi):
        return bi.ins if hasattr(bi, "ins") else bi

    f32 = mybir.dt.float32
    bf16 = mybir.dt.bfloat16
    AF = mybir.ActivationFunctionType

    nc = bacc.Bacc("TRN2", target_bir_lowering=False, debug=False,
                   num_devices=NCORES)

    xT = nc.declare_dram_parameter("xT", [E, T], bf16, isOutput=False)
    wq = nc.declare_dram_parameter("wq", [E, CLOC], bf16, isOutput=False)
    wk = nc.declare_dram_parameter("wk", [E, CLOC], bf16, isOutput=False)
    wv = nc.declare_dram_parameter("wv", [E, CLOC], bf16, isOutput=False)
    wo = nc.declare_dram_parameter("wo", [CLOC, E], bf16, isOutput=False)
    out = nc.declare_dram_parameter("out", [T, E], f32, isOutput=True)
    DBG = os.environ.get("KERNEL_DBG")
    dbg = (nc.declare_dram_parameter("dbg", [128, 8, 2048], bf16,
                                     isOutput=True)
           if DBG else None)

    KC = E // 128        # 8 contraction chunks for the projections
    NT = T // 128        # 16 T'-tiles (key rows per tile)
    NTC = T // 512       # 4 t-chunks (query columns per chunk)
    NPAIR = HLOC // 2    # 2 head pairs

    # PSUM budget (8 banks of 2KB/partition):
    #   bigps: shared ring of [128,2,512] tiles (scores, projections,
    #          out-proj) -> 3 bufs x 2 banks = 6 banks
    #   pvps:  2 per-head PV accumulators [65,512]     = 2 banks
    with tile.TileContext(nc) as tc:
        with (
            tc.tile_pool(name="persist", bufs=1) as persist,
            tc.tile_pool(name="bigps", bufs=3, space="PSUM") as bigps,
            tc.tile_pool(name="pvps", bufs=1, space="PSUM") as pvps,
            tc.tile_pool(name="ptile", bufs=6) as ptile,
            tc.tile_pool(name="small", bufs=2) as small,
            tc.tile_pool(name="ostg", bufs=3) as ostgp,
            tc.tile_pool(name="dscratch", bufs=4, space="DRAM") as dscratch,
        ):
            # ---- stage inputs in SBUF (all bf16) ----
            xT_sb = persist.tile([128, KC, T], bf16)
            wq_sb = persist.tile([128, KC, CLOC], bf16)
            wk_sb = persist.tile([128, KC, CLOC], bf16)
            wv_sb = persist.tile([128, KC, CLOC], bf16)
            xT_r = xT.ap().rearrange("(a p) t -> p a t", p=128)
            wq_r = wq.ap().rearrange("(a p) c -> p a c", p=128)
            wk_r = wk.ap().rearrange("(a p) c -> p a c", p=128)
            wv_r = wv.ap().rearrange("(a p) c -> p a c", p=128)
            # sync issues wk + the xT t-half each kc's first matmuls need,
            # interleaved and split by partition halves (same 2KB bursts,
            # more queue parallelism); gpsimd issues the rest; the scalar
            # engine issues nothing so exp starts unimpeded
            for kc in range(KC):
                nc.sync.dma_start(out=wk_sb[:, kc, :], in_=wk_r[:, kc, :])
                nc.sync.dma_start(
                    out=xT_sb[0:64, kc, 0:1024], in_=xT_r[0:64, kc, 0:1024]
                )
                nc.sync.dma_start(
                    out=xT_sb[64:128, kc, 0:1024],
                    in_=xT_r[64:128, kc, 0:1024],
                )
                nc.gpsimd.dma_start(out=wq_sb[:, kc, :], in_=wq_r[:, kc, :])
            for kc in range(KC):
                nc.gpsimd.dma_start(out=wv_sb[:, kc, :], in_=wv_r[:, kc, :])
                nc.sync.dma_start(
                    out=xT_sb[0:64, kc, 1024:T], in_=xT_r[0:64, kc, 1024:T]
                )
                nc.sync.dma_start(
                    out=xT_sb[64:128, kc, 1024:T],
                    in_=xT_r[64:128, kc, 1024:T],
                )
            # Wo rows for head pair pp live at partitions 0..127 of plane pp.
            wo_sb = persist.tile([128, HLOC // 2, E], bf16)
            wo_r = wo.ap().rearrange("(pp r) e -> r pp e", r=128)
            for pp in range(HLOC // 2):
                nc.gpsimd.dma_start(out=wo_sb[:, pp, :], in_=wo_r[:, pp, :])

            # ---- projections ----
            # Q^T, K^T: [CLOC, T] with head-local d on partitions
            # (M-group mg holds heads 2mg, 2mg+1).
            qT_sb = persist.tile([128, NPAIR, T], bf16)
            kT_sb = persist.tile([128, NPAIR, T], bf16)
            # V per t-tile: [t(128), tile, head, 65] with the ones column
            # for the softmax denominator at column 64 of each head slot.
            vp_sb = persist.tile([128, NT, HLOC, D + 1], bf16)
            nc.vector.memset(vp_sb[:, :, :, D : D + 1], 1.0)

            def proj_half(w_sb, dst, mg, tc2):
                """One projection group: head pair mg, t-half tc2. The two
                512-col halves evict separately so consumers unblock as soon
                as their half lands."""
                ps = bigps.tile([128, 2, 512], f32, tag="big")
                for half in range(2):
                    tcn = 2 * tc2 + half
                    for kc in range(KC):
                        nc.tensor.matmul(
                            ps[:, half, :],
                            lhsT=w_sb[:, kc, mg * 128 : (mg + 1) * 128],
                            rhs=xT_sb[:, kc, tcn * 512 : (tcn + 1) * 512],
                            start=(kc == 0),
                            stop=(kc == KC - 1),
                        )
                    nc.vector.tensor_copy(
                        out=dst[
                            :, mg, tc2 * 1024 + half * 512 :
                            tc2 * 1024 + (half + 1) * 512
                        ],
                        in_=ps[:, half, :],
                    )

            def v_proj_pair(tt0):
                # V tiles tt0, tt0+1 in natural [t, c] layout via x^T as the
                # stationary side; both share one ring slot and one eviction
                # copy so the jit projection doesn't thrash the scores ring
                ps = bigps.tile([128, 2, 512], f32, tag="big")
                for half in range(2):
                    tt = tt0 + half
                    for kc in range(KC):
                        nc.tensor.matmul(
                            ps[:, half, 0:CLOC],
                            lhsT=xT_sb[:, kc, tt * 128 : (tt + 1) * 128],
                            rhs=wv_sb[:, kc, :],
                            start=(kc == 0),
                            stop=(kc == KC - 1),
                        )
                nc.vector.tensor_copy(
                    out=vp_sb[:, tt0 : tt0 + 2, :, 0:D],
                    in_=ps[:, :, 0:CLOC].rearrange(
                        "p a (h d) -> p a h d", h=HLOC
                    ),
                )

            # headsN[c_lo, pair, t]: plane `pair` holds heads 2p (partitions
            # 0..63) and 2p+1 (64..127) — ready as out-proj stationary tiles.
            headsN = persist.tile([128, NPAIR, T], bf16)

            # per-chunk PV accumulators, live across the g-split
            pv_cur = [None, None]

            def attention(pair, tcn, g0, g1, jit_vproj=False):
                """Scores+exp+PV for T'-tile pairs g0..g1-1 of chunk
                (pair, tcn); caller finishes the chunk with attn_norm."""
                h0, h1 = 2 * pair, 2 * pair + 1
                tsl = slice(tcn * 512, (tcn + 1) * 512)
                if g0 == 0:
                    pv_cur[0] = pvps.tile([65, 512], f32, name="pv0", tag="pv0")
                    pv_cur[1] = pvps.tile([65, 512], f32, name="pv1", tag="pv1")
                pv0, pv1 = pv_cur
                for g in range(g0, g1):
                    st = (g == 0)
                    sp = (g == NT // 2 - 1)
                    pss, pts = [], []
                    last_s = None
                    for i in range(2):
                        tt = 2 * g + i
                        ksl = slice(tt * 128, (tt + 1) * 128)
                        ps_s = bigps.tile([128, 2, 512], f32, tag="big")
                        pss.append(ps_s)
                        # S^T for both heads (row-packed on the PE)
                        nc.tensor.matmul(
                            ps_s[:, 0, :],
                            lhsT=kT_sb[0:64, pair, ksl],
                            rhs=qT_sb[0:64, pair, tsl],
                            tile_position=(0, 0),
                        )
                        last_s = nc.tensor.matmul(
                            ps_s[:, 1, :],
                            lhsT=kT_sb[64:128, pair, ksl],
                            rhs=qT_sb[64:128, pair, tsl],
                            tile_position=(64, 0),
                        )
                    for i in range(2):
                        pt = ptile.tile([128, 2, 512], bf16, tag="pt")
                        pts.append(pt)
                        nc.scalar.activation(
                            out=pt[:], in_=pss[i][:], func=AF.Exp,
                            scale=0.125,
                        )
                    if jit_vproj:
                        # first consumer of these V tiles: project them
                        # now so the PE fills exp-wait slack
                        v_proj_pair(2 * g)
                    for i in range(2):
                        tt = 2 * g + i
                        mm = nc.tensor.matmul(
                            pv0[:, :],
                            lhsT=vp_sb[:, tt, h0, :],
                            rhs=pts[i][:, 0, :],
                            start=(st and i == 0), stop=(sp and i == 1),
                        )
                        if i == 0:
                            # keep the two scores groups adjacent on the
                            # PE (same-config runs pipeline; interleaved
                            # configs pay the full isolated matmul cost)
                            add_dep_helper(
                                _inst(mm), _inst(last_s),
                                reason="batch scores before pv",
                            )
                        nc.tensor.matmul(
                            pv1[:, :],
                            lhsT=vp_sb[:, tt, h1, :],
                            rhs=pts[i][:, 1, :],
                            start=(st and i == 0), stop=(sp and i == 1),
                        )

            def attn_norm(pair, tcn, fine=False):
                """Finish chunk (pair, tcn): evacuate the two per-head PV
                accumulators, build the broadcast reciprocal denominator,
                normalize into headsN. h1's block is DMA-shifted to
                partitions 64..127. `fine` splits the multiplies per
                128-col out-proj tile to shorten the end-of-kernel tail."""
                tsl = slice(tcn * 512, (tcn + 1) * 512)
                pv0, pv1 = pv_cur
                stgA = small.tile([65, 512], f32, tag="stgA")
                nc.vector.tensor_copy(out=stgA[:], in_=pv0[:])
                stgB = small.tile([65, 512], f32, tag="stgB")
                nc.vector.tensor_copy(out=stgB[:], in_=pv1[:])
                # bounce the raw denominator rows through DRAM to
                # broadcast along partitions (stride-0 SBUF DMA sources are
                # illegal), then reciprocal on the [64,512] broadcast.
                # NB: reciprocal_approx_fast is a custom-DVE op; running it
                # in-place on a [1,512] single-partition row corrupts
                # unrelated SBUF -- keep it out-of-place on the broadcast.
                dscA = dscratch.tile([1, 512], f32, tag="dscA")
                nc.sync.dma_start(out=dscA[:], in_=stgA[64:65, :])
                dscB = dscratch.tile([1, 512], f32, tag="dscB")
                nc.gpsimd.dma_start(out=dscB[:], in_=stgB[64:65, :])
                den0 = small.tile([64, 512], f32, tag="den0")
                nc.sync.dma_start(
                    out=den0[:], in_=dscA[:].to_broadcast([64, 512])
                )
                den1 = small.tile([64, 512], f32, tag="den1")
                nc.sync.dma_start(
                    out=den1[:], in_=dscB[:].to_broadcast([64, 512])
                )
                rec0 = small.tile([64, 512], f32, tag="rec0")
                nc.vector.reciprocal_approx_fast(out=rec0[:], in_=den0[:])
                rec1 = small.tile([64, 512], f32, tag="rec1")
                nc.vector.reciprocal_approx_fast(out=rec1[:], in_=den1[:])
                cols = 4 if fine else 1
                w = 512 // cols
                for j in range(cols):
                    jsl = slice(j * w, (j + 1) * w)
                    hsl = slice(tcn * 512 + j * w, tcn * 512 + (j + 1) * w)
                    nc.vector.tensor_mul(
                        out=headsN[0:64, pair, hsl],
                        in0=stgA[0:64, jsl], in1=rec0[:, jsl],
                    )
                    tmpB = small.tile([64, 512], bf16, tag="tmpB")
                    nc.vector.tensor_mul(
                        out=tmpB[:, jsl], in0=stgB[0:64, jsl],
                        in1=rec1[:, jsl],
                    )
                    nc.sync.dma_start(
                        out=headsN[64:128, pair, hsl], in_=tmpB[:, jsl]
                    )

            def out_tile(tt):
                # partial[t-tile] = headsN^T @ Wo_loc ; both e-chunks share
                # one PSUM tile; staging copy on gpsimd keeps vector free
                ksl = slice(tt * 128, (tt + 1) * 128)
                ps_o = bigps.tile([128, 2, 512], f32, tag="big")
                for ec in range(2):
                    esl = slice(ec * 512, (ec + 1) * 512)
                    for pp in range(NPAIR):
                        nc.tensor.matmul(
                            ps_o[:, ec, :],
                            lhsT=headsN[:, pp, ksl],
                            rhs=wo_sb[:, pp, esl],
                            start=(pp == 0),
                            stop=(pp == NPAIR - 1),
                        )
                stg = ostgp.tile([128, 2, 512], f32, tag="ostg")
                nc.vector.tensor_copy(out=stg[:], in_=ps_o[:])
                nc.sync.dma_start(
                    out=out.ap()[ksl, :],
                    in_=stg[:].rearrange("p a b -> p (a b)"),
                )

            # Emission order IS program order (Tile tracks deps in trace
            # order) and doubles as scheduler priority. Only what the next
            # attention piece needs is projected first (scores for tile
            # pairs g0..3 want just the first K t-half and the current Q
            # chunk) so the scalar engine starts exp as early as possible;
            # V tiles are projected just-in-time inside pair-0 tcn-0;
            # out-proj tiles stream after each pair-1 chunk.
            for pair in range(NPAIR):
                proj_half(wk_sb, kT_sb, pair, 0)
                proj_half(wq_sb, qT_sb, pair, 0)
                attention(pair, 0, 0, 4, jit_vproj=(pair == 0))
                proj_half(wk_sb, kT_sb, pair, 1)
                attention(pair, 0, 4, 8, jit_vproj=(pair == 0))
                attn_norm(pair, 0)
                attention(pair, 1, 0, 8)
                attn_norm(pair, 1)
                if pair == 1:
                    for tt in range(0, 8):
                        out_tile(tt)
                proj_half(wq_sb, qT_sb, pair, 1)
                attention(pair, 2, 0, 8)
                attn_norm(pair, 2)
                if pair == 1:
                    for tt in range(8, 12):
                        out_tile(tt)
                attention(pair, 3, 0, 8)
                attn_norm(pair, 3, fine=(pair == 1))
                if pair == 1:
                    for tt in range(12, 16):
                        out_tile(tt)
            if dbg is not None:
                if DBG == "xt":
                    for kc in range(KC):
                        nc.sync.dma_start(
                            out=dbg.ap()[:, kc, :], in_=xT_sb[:, kc, :]
                        )
                elif DBG == "qt":
                    nc.sync.dma_start(out=dbg.ap()[:, 0, :], in_=qT_sb[:, 0, :])
                    nc.sync.dma_start(out=dbg.ap()[:, 1, :], in_=qT_sb[:, 1, :])
                    nc.sync.dma_start(out=dbg.ap()[:, 2, :], in_=kT_sb[:, 0, :])
                    nc.sync.dma_start(out=dbg.ap()[:, 3, :], in_=kT_sb[:, 1, :])
                elif DBG == "heads":
                    nc.sync.dma_start(out=dbg.ap()[:, 0, :], in_=headsN[:, 0, :])
                    nc.sync.dma_start(out=dbg.ap()[:, 1, :], in_=headsN[:, 1, :])

    nc.compile()
    return nc


def _get_program():
    global _cached_nc
    if _cached_nc is None:
        _cached_nc = _build_program()
    return _cached_nc


def kernel(x, Wq, Wk, Wv, Wo):
    global LAST_EXEC_NS
    _ensure_ntff_hook()
    from concourse.bass_utils import run_bass_kernel_spmd

    nc = _get_program()
    bf16 = ml_dtypes.bfloat16

    x = np.asarray(x, dtype=np.float32)
    in_maps = []
    for c in range(NCORES):
        b = c // TPC
        hs = (c % TPC) * HLOC
        xT_c = np.ascontiguousarray(x[b].T).astype(bf16)
        # [HLOC, E, D] -> [E, HLOC*D]
        wq_c = np.ascontiguousarray(
            np.asarray(Wq)[hs : hs + HLOC].transpose(1, 0, 2).reshape(E, CLOC)
        ).astype(bf16)
        wk_c = np.ascontiguousarray(
            np.asarray(Wk)[hs : hs + HLOC].transpose(1, 0, 2).reshape(E, CLOC)
        ).astype(bf16)
        wv_c = np.ascontiguousarray(
            np.asarray(Wv)[hs : hs + HLOC].transpose(1, 0, 2).reshape(E, CLOC)
        ).astype(bf16)
        wo_c = np.ascontiguousarray(
            np.asarray(Wo)[hs * D : (hs + HLOC) * D, :]
        ).astype(bf16)
        in_maps.append(
            {"xT": xT_c, "wq": wq_c, "wk": wk_c, "wv": wv_c, "wo": wo_c}
        )

    trace = bool(os.environ.get("KERNEL_TRACE"))
    res = run_bass_kernel_spmd(nc, in_maps, list(range(NCORES)), trace=trace)
    LAST_EXEC_NS = res.exec_time_ns

    out = np.empty((B, T, E), dtype=np.float32)
    for b in range(B):
        acc = x[b].copy()
        for g in range(TPC):
            acc += res.results[b * TPC + g]["out"]
        out[b] = acc
    return out


# revision 18
# speedup vs baseline: 1.0899x; 1.0798x over previous
"""Multi-head attention Trainium2 kernel (8 NeuronCores).

Problem: B=2, T=2048, E=1024, H=16, D=64 multi-head attention
    q/k/v = einsum('bte,hed->bhtd', x, W{q,k,v})
    out   = softmax(q k^T / sqrt(D)) v, heads concat, @ Wo, + x

Sharding: data-parallel over batch (2 groups of 4 cores) x tensor-parallel
over heads (4 heads per core). Each core computes, for its batch b and its
4 heads, the partial output  partial = concat_heads(attn) @ Wo[head rows].
The host sums the 4 partials per batch and adds the residual x.

Device layout notes:
  - All matmul inputs are bf16 (fp32 PSUM accumulation); exp runs on the
    scalar engine in fp32 reading scores straight from PSUM.
  - Scores are computed transposed, S^T[T', t], so that P^T = exp(S^T)
    lands directly in the layout the PV matmul needs as its moving
    operand (contraction over T' on partitions).
  - The softmax denominator rides along in the PV matmul: each head's V
    tile carries a ones column ([V_h | 1], M=65), so PSUM rows 0..63 of
    the per-head accumulator are (P V)^T and row 64 is sum(P). This
    replaces the v1 M=1 denominator matmuls (256 x ~160ns of pure PE
    column-streaming).
  - Each head accumulates in its own [65,512] PSUM bank (65+65 > 128
    partitions, so the two heads can no longer be column-packed). The
    h1 block is moved to partitions 64..127 of headsN by a small
    SBUF->SBUF DMA (compute engines cannot shift partitions; DMA can).
  - The denominator reciprocal is taken on the [1,512] row, then
    DMA-broadcast along partitions.
  - exp is not max-subtracted: scores/8 lie in roughly [-10, 10] for this
    problem family, far inside fp32 exp range.
  - out-proj tiles are interleaved with pair-1 attention chunks so the
    output DMA streams throughout instead of draining at the end.
"""

import contextlib
import ctypes
import os
import sys
import types

import numpy as np
import ml_dtypes

B, T, E, H = 2, 2048, 1024, 16
D = E // H          # 64
NCORES = 8
DP = 2              # batch groups
TPC = NCORES // DP  # cores per batch group
HLOC = H // TPC     # heads per core = 4
CLOC = HLOC * D     # local concat width = 256

_cached_nc = None
LAST_EXEC_NS = None


def _ensure_ntff_hook():
    """bass_utils' trace path imports antenv.axon_hooks, which is absent in
    this image. Recreate it (registry + ctypes NTFF driver) so profiled runs
    don't crash; no-op if the module already exists."""
    try:
        import antenv.axon_hooks  # noqa: F401
        return
    except ImportError:
        pass
    try:
        import antenv
    except ImportError:
        return

    mod = types.ModuleType("antenv.axon_hooks")
    _state = {"hook": None}
    mod.set_axon_ntff_profile_hook = lambda h: _state.__setitem__("hook", h)
    mod.get_axon_ntff_profile_hook = lambda: _state["hook"]
    sys.modules["antenv.axon_hooks"] = mod
    antenv.axon_hooks = mod

    so_path = "/opt/axon/libaxon_pjrt.so"
    if not os.path.exists(so_path):
        return
    try:
        lib = ctypes.CDLL(so_path)
    except OSError:
        return
    if not hasattr(lib, "axon_start_nrt_profile"):
        return
    lib.axon_start_nrt_profile.argtypes = [
        ctypes.POINTER(ctypes.c_int64),
        ctypes.c_size_t,
    ]
    lib.axon_start_nrt_profile.restype = ctypes.c_int64
    lib.axon_stop_nrt_profile.argtypes = [ctypes.c_char_p]
    lib.axon_stop_nrt_profile.restype = ctypes.c_int64

    @contextlib.contextmanager
    def _hook(output_dir, device_ids):
        import jax

        jax.devices()
        if device_ids:
            ids = (ctypes.c_int64 * len(device_ids))(*device_ids)
            rc = lib.axon_start_nrt_profile(ids, len(device_ids))
        else:
            rc = lib.axon_start_nrt_profile(None, 0)
        if rc != 0:
            raise RuntimeError(f"axon_start_nrt_profile rc={rc}")
        try:
            yield
        finally:
            n = lib.axon_stop_nrt_profile(str(output_dir).encode())
            print(f"ntff profile: {n} file(s) -> {output_dir}", file=sys.stderr)

    mod.set_axon_ntff_profile_hook(_hook)


def _build_program():
    import concourse.mybir as mybir
    import concourse.tile as tile
    from concourse import bacc
    from concourse.tile_rust import add_dep_helper

    def _inst(bi):
        return bi.ins if hasattr(bi, "ins") else bi

    f32 = mybir.dt.float32
    bf16 = mybir.dt.bfloat16
    AF = mybir.ActivationFunctionType

    nc = bacc.Bacc("TRN2", target_bir_lowering=False, debug=False,
                   num_devices=NCORES)

    xT = nc.declare_dram_parameter("xT", [E, T], bf16, isOutput=False)
    wq = nc.declare_dram_parameter("wq", [E, CLOC], bf16, isOutput=False)
    wk = nc.declare_dram_parameter("wk", [E, CLOC], bf16, isOutput=False)
    wv = nc.declare_dram_parameter("wv", [E, CLOC], bf16, isOutput=False)
    wo = nc.declare_dram_parameter("wo", [CLOC, E], bf16, isOutput=False)
    out = nc.declare_dram_parameter("out", [T, E], f32, isOutput=True)
    DBG = os.environ.get("KERNEL_DBG")
    dbg = (nc.declare_dram_parameter("dbg", [128, 8, 2048], bf16,
                                     isOutput=True)
           if DBG else None)

    KC = E // 128        # 8 contraction chunks for the projections
    NT = T // 128        # 16 T'-tiles (key rows per tile)
    NTC = T // 512       # 4 t-chunks (query columns per chunk)
    NPAIR = HLOC // 2    # 2 head pairs

    # PSUM budget (8 banks of 2KB/partition):
    #   bigps: shared ring of [128,2,512] tiles (scores, projections,
    #          out-proj) -> 3 bufs x 2 banks = 6 banks
    #   pvps:  2 per-head PV accumulators [65,512]     = 2 banks
    with tile.TileContext(nc) as tc:
        with (
            tc.tile_pool(name="persist", bufs=1) as persist,
            tc.tile_pool(name="bigps", bufs=3, space="PSUM") as bigps,
            tc.tile_pool(name="pvps", bufs=1, space="PSUM") as pvps,
            tc.tile_pool(name="ptile", bufs=6) as ptile,
            tc.tile_pool(name="small", bufs=2) as small,
            tc.tile_pool(name="ostg", bufs=3) as ostgp,
            tc.tile_pool(name="dscratch", bufs=4, space="DRAM") as dscratch,
        ):
            # ---- stage inputs in SBUF (all bf16) ----
            xT_sb = persist.tile([128, KC, T], bf16)
            wq_sb = persist.tile([128, KC, CLOC], bf16)
            wk_sb = persist.tile([128, KC, CLOC], bf16)
            wv_sb = persist.tile([128, KC, CLOC], bf16)
            xT_r = xT.ap().rearrange("(a p) t -> p a t", p=128)
            wq_r = wq.ap().rearrange("(a p) c -> p a c", p=128)
            wk_r = wk.ap().rearrange("(a p) c -> p a c", p=128)
            wv_r = wv.ap().rearrange("(a p) c -> p a c", p=128)
            # sync issues wk + the xT t-half each kc's first matmuls need,
            # interleaved and split by partition halves (same 2KB bursts,
            # more queue parallelism); gpsimd issues the rest; the scalar
            # engine issues nothing so exp starts unimpeded
            for kc in range(KC):
                nc.sync.dma_start(out=wk_sb[:, kc, :], in_=wk_r[:, kc, :])
                nc.sync.dma_start(
                    out=xT_sb[0:64, kc, 0:1024], in_=xT_r[0:64, kc, 0:1024]
                )
                nc.sync.dma_start(
                    out=xT_sb[64:128, kc, 0:1024],
                    in_=xT_r[64:128, kc, 0:1024],
                )
                nc.gpsimd.dma_start(out=wq_sb[:, kc, :], in_=wq_r[:, kc, :])
            for kc in range(KC):
                nc.gpsimd.dma_start(out=wv_sb[:, kc, :], in_=wv_r[:, kc, :])
                nc.sync.dma_start(
                    out=xT_sb[0:64, kc, 1024:T], in_=xT_r[0:64, kc, 1024:T]
                )
                nc.sync.dma_start(
                    out=xT_sb[64:128, kc, 1024:T],
                    in_=xT_r[64:128, kc, 1024:T],
                )
            # Wo rows for head pair pp live at partitions 0..127 of plane pp.
            wo_sb = persist.tile([128, HLOC // 2, E], bf16)
            wo_r = wo.ap().rearrange("(pp r) e -> r pp e", r=128)
            for pp in range(HLOC // 2):
                nc.gpsimd.dma_start(out=wo_sb[:, pp, :], in_=wo_r[:, pp, :])

            # ---- projections ----
            # Q^T, K^T: [CLOC, T] with head-local d on partitions
            # (M-group mg holds heads 2mg, 2mg+1).
            qT_sb = persist.tile([128, NPAIR, T], bf16)
            kT_sb = persist.tile([128, NPAIR, T], bf16)
            # V per t-tile: [t(128), tile, head, 65] with the ones column
            # for the softmax denominator at column 64 of each head slot.
            vp_sb = persist.tile([128, NT, HLOC, D + 1], bf16)
            nc.vector.memset(vp_sb[:, :, :, D : D + 1], 1.0)

            def proj_half(w_sb, dst, mg, tc2):
                """One projection group: head pair mg, t-half tc2. The two
                512-col halves evict separately so consumers unblock as soon
                as their half lands."""
                ps = bigps.tile([128, 2, 512], f32, tag="big")
                for half in range(2):
                    tcn = 2 * tc2 + half
                    for kc in range(KC):
                        nc.tensor.matmul(
                            ps[:, half, :],
                            lhsT=w_sb[:, kc, mg * 128 : (mg + 1) * 128],
                            rhs=xT_sb[:, kc, tcn * 512 : (tcn + 1) * 512],
                            start=(kc == 0),
                            stop=(kc == KC - 1),
                        )
                    nc.vector.tensor_copy(
                        out=dst[
                            :, mg, tc2 * 1024 + half * 512 :
                            tc2 * 1024 + (half + 1) * 512
                        ],
                        in_=ps[:, half, :],
                    )

            def v_proj_pair(tt0):
                # V tiles tt0, tt0+1 in natural [t, c] layout via x^T as the
                # stationary side; both share one ring slot and one eviction
                # copy so the jit projection doesn't thrash the scores ring
                ps = bigps.tile([128, 2, 512], f32, tag="big")
                for half in range(2):
                    tt = tt0 + half
                    for kc in range(KC):
                        nc.tensor.matmul(
                            ps[:, half, 0:CLOC],
                            lhsT=xT_sb[:, kc, tt * 128 : (tt + 1) * 128],
                            rhs=wv_sb[:, kc, :],
                            start=(kc == 0),
                            stop=(kc == KC - 1),
                        )
                nc.vector.tensor_copy(
                    out=vp_sb[:, tt0 : tt0 + 2, :, 0:D],
                    in_=ps[:, :, 0:CLOC].rearrange(
                        "p a (h d) -> p a h d", h=HLOC
                    ),
                )

            # headsN[c_lo, pair, t]: plane `pair` holds heads 2p (partitions
            # 0..63) and 2p+1 (64..127) — ready as out-proj stationary tiles.
            headsN = persist.tile([128, NPAIR, T], bf16)

            # per-chunk PV accumulators, live across the g-split
            pv_cur = [None, None]

            def attention(pair, tcn, g0, g1, jit_vproj=False):
                """Scores+exp+PV for T'-tile pairs g0..g1-1 of chunk
                (pair, tcn); caller finishes the chunk with attn_norm."""
                h0, h1 = 2 * pair, 2 * pair + 1
                tsl = slice(tcn * 512, (tcn + 1) * 512)
                if g0 == 0:
                    pv_cur[0] = pvps.tile([65, 512], f32, name="pv0", tag="pv0")
                    pv_cur[1] = pvps.tile([65, 512], f32, name="pv1", tag="pv1")
                pv0, pv1 = pv_cur
                for g in range(g0, g1):
                    st = (g == 0)
                    sp = (g == NT // 2 - 1)
                    pss, pts = [], []
                    last_s = None
                    for i in range(2):
                        tt = 2 * g + i
                        ksl = slice(tt * 128, (tt + 1) * 128)
                        ps_s = bigps.tile([128, 2, 512], f32, tag="big")
                        pss.append(ps_s)
                        # S^T for both heads (row-packed on the PE)
                        nc.tensor.matmul(
                            ps_s[:, 0, :],
                            lhsT=kT_sb[0:64, pair, ksl],
                            rhs=qT_sb[0:64, pair, tsl],
                            tile_position=(0, 0),
                        )
                        last_s = nc.tensor.matmul(
                            ps_s[:, 1, :],
                            lhsT=kT_sb[64:128, pair, ksl],
                            rhs=qT_sb[64:128, pair, tsl],
                            tile_position=(64, 0),
                        )
                    for i in range(2):
                        pt = ptile.tile([128, 2, 512], bf16, tag="pt")
                        pts.append(pt)
                        nc.scalar.activation(
                            out=pt[:], in_=pss[i][:], func=AF.Exp,
                            scale=0.125,
                        )
                    if jit_vproj:
                        # first consumer of these V tiles: project them
                        # now so the PE fills exp-wait slack
                        v_proj_pair(2 * g)
                    for i in range(2):
                        tt = 2 * g + i
                        mm = nc.tensor.matmul(
                            pv0[:, :],
                            lhsT=vp_sb[:, tt, h0, :],
                            rhs=pts[i][:, 0, :],
                            start=(st and i == 0), stop=(sp and i == 1),
                        )
                        if i == 0:
                            # keep the two scores groups adjacent on the
                            # PE (same-config runs pipeline; interleaved
                            # configs pay the full isolated matmul cost)
                            add_dep_helper(
                                _inst(mm), _inst(last_s),
                                reason="batch scores before pv",
                            )
                        nc.tensor.matmul(
                            pv1[:, :],
                            lhsT=vp_sb[:, tt, h1, :],
                            rhs=pts[i][:, 1, :],
                            start=(st and i == 0), stop=(sp and i == 1),
                        )

            def attn_norm(pair, tcn, fine=False):
                """Finish chunk (pair, tcn): evacuate the two per-head PV
                accumulators, build the broadcast reciprocal denominator,
                normalize into headsN. h1's block is DMA-shifted to
                partitions 64..127. `fine` splits the multiplies per
                128-col out-proj tile to shorten the end-of-kernel tail."""
                tsl = slice(tcn * 512, (tcn + 1) * 512)
                pv0, pv1 = pv_cur
                stgA = small.tile([65, 512], f32, tag="stgA")
                nc.vector.tensor_copy(out=stgA[:], in_=pv0[:])
                stgB = small.tile([65, 512], f32, tag="stgB")
                nc.vector.tensor_copy(out=stgB[:], in_=pv1[:])
                # bounce the raw denominator rows through DRAM to
                # broadcast along partitions (stride-0 SBUF DMA sources and
                # gpsimd partition_broadcast from a base-64 row both fail).
                # NB: reciprocal_approx_fast is a custom-DVE op; running it
                # in-place on a [1,512] single-partition row corrupts
                # unrelated SBUF -- keep it out-of-place on the broadcast.
                dscA = dscratch.tile([1, 512], f32, tag="dscA")
                nc.sync.dma_start(out=dscA[:], in_=stgA[64:65, :])
                dscB = dscratch.tile([1, 512], f32, tag="dscB")
                nc.gpsimd.dma_start(out=dscB[:], in_=stgB[64:65, :])
                den0 = small.tile([64, 512], f32, tag="den0")
                nc.sync.dma_start(
                    out=den0[:], in_=dscA[:].to_broadcast([64, 512])
                )
                den1 = small.tile([64, 512], f32, tag="den1")
                nc.sync.dma_start(
                    out=den1[:], in_=dscB[:].to_broadcast([64, 512])
                )
                rec0 = small.tile([64, 512], f32, tag="rec0")
                nc.vector.reciprocal_approx_fast(out=rec0[:], in_=den0[:])
                rec1 = small.tile([64, 512], f32, tag="rec1")
                nc.vector.reciprocal_approx_fast(out=rec1[:], in_=den1[:])
                cols = 4 if fine else 1
                w = 512 // cols
                for j in range(cols):
                    jsl = slice(j * w, (j + 1) * w)
                    hsl = slice(tcn * 512 + j * w, tcn * 512 + (j + 1) * w)
                    nc.vector.tensor_mul(
                        out=headsN[0:64, pair, hsl],
                        in0=stgA[0:64, jsl], in1=rec0[:, jsl],
                    )
                    tmpB = small.tile([64, 512], bf16, tag="tmpB")
                    nc.vector.tensor_mul(
                        out=tmpB[:, jsl], in0=stgB[0:64, jsl],
                        in1=rec1[:, jsl],
                    )
                    nc.sync.dma_start(
                        out=headsN[64:128, pair, hsl], in_=tmpB[:, jsl]
                    )

            def out_tile(tt):
                # partial[t-tile] = headsN^T @ Wo_loc ; both e-chunks share
                # one PSUM tile; staging copy on gpsimd keeps vector free
                ksl = slice(tt * 128, (tt + 1) * 128)
                ps_o = bigps.tile([128, 2, 512], f32, tag="big")
                for ec in range(2):
                    esl = slice(ec * 512, (ec + 1) * 512)
                    for pp in range(NPAIR):
                        nc.tensor.matmul(
                            ps_o[:, ec, :],
                            lhsT=headsN[:, pp, ksl],
                            rhs=wo_sb[:, pp, esl],
                            start=(pp == 0),
                            stop=(pp == NPAIR - 1),
                        )
                stg = ostgp.tile([128, 2, 512], f32, tag="ostg")
                nc.vector.tensor_copy(out=stg[:], in_=ps_o[:])
                nc.sync.dma_start(
                    out=out.ap()[ksl, :],
                    in_=stg[:].rearrange("p a b -> p (a b)"),
                )

            # Emission order IS program order (Tile tracks deps in trace
            # order) and doubles as scheduler priority. Only what the next
            # attention piece needs is projected first (scores for tile
            # pairs g0..3 want just the first K t-half and the current Q
            # chunk) so the scalar engine starts exp as early as possible;
            # V tiles are projected just-in-time inside pair-0 tcn-0;
            # out-proj tiles stream after each pair-1 chunk.
            for pair in range(NPAIR):
                proj_half(wk_sb, kT_sb, pair, 0)
                proj_half(wq_sb, qT_sb, pair, 0)
                attention(pair, 0, 0, 4, jit_vproj=(pair == 0))
                proj_half(wk_sb, kT_sb, pair, 1)
                attention(pair, 0, 4, 8, jit_vproj=(pair == 0))
                attn_norm(pair, 0)
                attention(pair, 1, 0, 8)
                attn_norm(pair, 1)
                if pair == 1:
                    # one-chunk lag: when the PE reaches these, the norm
                    # chain they depend on completed a whole chunk ago
                    for tt in range(0, 4):
                        out_tile(tt)
                proj_half(wq_sb, qT_sb, pair, 1)
                attention(pair, 2, 0, 8)
                attn_norm(pair, 2)
                if pair == 1:
                    for tt in range(4, 8):
                        out_tile(tt)
                attention(pair, 3, 0, 8)
                attn_norm(pair, 3, fine=(pair == 1))
                if pair == 1:
                    for tt in range(8, 16):
                        out_tile(tt)
            if dbg is not None:
                if DBG == "xt":
                    for kc in range(KC):
                        nc.sync.dma_start(
                            out=dbg.ap()[:, kc, :], in_=xT_sb[:, kc, :]
                        )
                elif DBG == "qt":
                    nc.sync.dma_start(out=dbg.ap()[:, 0, :], in_=qT_sb[:, 0, :])
                    nc.sync.dma_start(out=dbg.ap()[:, 1, :], in_=qT_sb[:, 1, :])
                    nc.sync.dma_start(out=dbg.ap()[:, 2, :], in_=kT_sb[:, 0, :])
                    nc.sync.dma_start(out=dbg.ap()[:, 3, :], in_=kT_sb[:, 1, :])
                elif DBG == "heads":
                    nc.sync.dma_start(out=dbg.ap()[:, 0, :], in_=headsN[:, 0, :])
                    nc.sync.dma_start(out=dbg.ap()[:, 1, :], in_=headsN[:, 1, :])

    nc.compile()
    return nc


def _get_program():
    global _cached_nc
    if _cached_nc is None:
        _cached_nc = _build_program()
    return _cached_nc


def kernel(x, Wq, Wk, Wv, Wo):
    global LAST_EXEC_NS
    _ensure_ntff_hook()
    from concourse.bass_utils import run_bass_kernel_spmd

    nc = _get_program()
    bf16 = ml_dtypes.bfloat16

    x = np.asarray(x, dtype=np.float32)
    in_maps = []
    for c in range(NCORES):
        b = c // TPC
        hs = (c % TPC) * HLOC
        xT_c = np.ascontiguousarray(x[b].T).astype(bf16)
        # [HLOC, E, D] -> [E, HLOC*D]
        wq_c = np.ascontiguousarray(
            np.asarray(Wq)[hs : hs + HLOC].transpose(1, 0, 2).reshape(E, CLOC)
        ).astype(bf16)
        wk_c = np.ascontiguousarray(
            np.asarray(Wk)[hs : hs + HLOC].transpose(1, 0, 2).reshape(E, CLOC)
        ).astype(bf16)
        wv_c = np.ascontiguousarray(
            np.asarray(Wv)[hs : hs + HLOC].transpose(1, 0, 2).reshape(E, CLOC)
        ).astype(bf16)
        wo_c = np.ascontiguousarray(
            np.asarray(Wo)[hs * D : (hs + HLOC) * D, :]
        ).astype(bf16)
        in_maps.append(
            {"xT": xT_c, "wq": wq_c, "wk": wk_c, "wv": wv_c, "wo": wo_c}
        )

    trace = bool(os.environ.get("KERNEL_TRACE"))
    res = run_bass_kernel_spmd(nc, in_maps, list(range(NCORES)), trace=trace)
    LAST_EXEC_NS = res.exec_time_ns

    out = np.empty((B, T, E), dtype=np.float32)
    for b in range(B):
        acc = x[b].copy()
        for g in range(TPC):
            acc += res.results[b * TPC + g]["out"]
        out[b] = acc
    return out


# revision 19
# speedup vs baseline: 1.1067x; 1.0154x over previous
"""Multi-head attention Trainium2 kernel (8 NeuronCores).

Problem: B=2, T=2048, E=1024, H=16, D=64 multi-head attention
    q/k/v = einsum('bte,hed->bhtd', x, W{q,k,v})
    out   = softmax(q k^T / sqrt(D)) v, heads concat, @ Wo, + x

Sharding: data-parallel over batch (2 groups of 4 cores) x tensor-parallel
over heads (4 heads per core). Each core computes, for its batch b and its
4 heads, the partial output  partial = concat_heads(attn) @ Wo[head rows].
The host sums the 4 partials per batch and adds the residual x.

Device layout notes:
  - All matmul inputs are bf16 (fp32 PSUM accumulation); exp runs on the
    scalar engine in fp32 reading scores straight from PSUM.
  - Scores are computed transposed, S^T[T', t], so that P^T = exp(S^T)
    lands directly in the layout the PV matmul needs as its moving
    operand (contraction over T' on partitions).
  - The softmax denominator rides along in the PV matmul: each head's V
    tile carries a ones column ([V_h | 1], M=65), so PSUM rows 0..63 of
    the per-head accumulator are (P V)^T and row 64 is sum(P). This
    replaces the v1 M=1 denominator matmuls (256 x ~160ns of pure PE
    column-streaming).
  - Each head accumulates in its own [65,512] PSUM bank (65+65 > 128
    partitions, so the two heads can no longer be column-packed). The
    h1 block is moved to partitions 64..127 of headsN by a small
    SBUF->SBUF DMA (compute engines cannot shift partitions; DMA can).
  - The denominator reciprocal is taken on the [1,512] row, then
    DMA-broadcast along partitions.
  - exp is not max-subtracted: scores/8 lie in roughly [-10, 10] for this
    problem family, far inside fp32 exp range.
  - out-proj tiles are interleaved with pair-1 attention chunks so the
    output DMA streams throughout instead of draining at the end.
"""

import contextlib
import ctypes
import os
import sys
import types

import numpy as np
import ml_dtypes

B, T, E, H = 2, 2048, 1024, 16
D = E // H          # 64
NCORES = 8
DP = 2              # batch groups
TPC = NCORES // DP  # cores per batch group
HLOC = H // TPC     # heads per core = 4
CLOC = HLOC * D     # local concat width = 256

_cached_nc = None
LAST_EXEC_NS = None


def _ensure_ntff_hook():
    """bass_utils' trace path imports antenv.axon_hooks, which is absent in
    this image. Recreate it (registry + ctypes NTFF driver) so profiled runs
    don't crash; no-op if the module already exists."""
    try:
        import antenv.axon_hooks  # noqa: F401
        return
    except ImportError:
        pass
    try:
        import antenv
    except ImportError:
        return

    mod = types.ModuleType("antenv.axon_hooks")
    _state = {"hook": None}
    mod.set_axon_ntff_profile_hook = lambda h: _state.__setitem__("hook", h)
    mod.get_axon_ntff_profile_hook = lambda: _state["hook"]
    sys.modules["antenv.axon_hooks"] = mod
    antenv.axon_hooks = mod

    so_path = "/opt/axon/libaxon_pjrt.so"
    if not os.path.exists(so_path):
        return
    try:
        lib = ctypes.CDLL(so_path)
    except OSError:
        return
    if not hasattr(lib, "axon_start_nrt_profile"):
        return
    lib.axon_start_nrt_profile.argtypes = [
        ctypes.POINTER(ctypes.c_int64),
        ctypes.c_size_t,
    ]
    lib.axon_start_nrt_profile.restype = ctypes.c_int64
    lib.axon_stop_nrt_profile.argtypes = [ctypes.c_char_p]
    lib.axon_stop_nrt_profile.restype = ctypes.c_int64

    @contextlib.contextmanager
    def _hook(output_dir, device_ids):
        import jax

        jax.devices()
        if device_ids:
            ids = (ctypes.c_int64 * len(device_ids))(*device_ids)
            rc = lib.axon_start_nrt_profile(ids, len(device_ids))
        else:
            rc = lib.axon_start_nrt_profile(None, 0)
        if rc != 0:
            raise RuntimeError(f"axon_start_nrt_profile rc={rc}")
        try:
            yield
        finally:
            n = lib.axon_stop_nrt_profile(str(output_dir).encode())
            print(f"ntff profile: {n} file(s) -> {output_dir}", file=sys.stderr)

    mod.set_axon_ntff_profile_hook(_hook)


def _build_program():
    import concourse.mybir as mybir
    import concourse.tile as tile
    from concourse import bacc
    from concourse.tile_rust import add_dep_helper

    def _inst(bi):
        return bi.ins if hasattr(bi, "ins") else bi

    f32 = mybir.dt.float32
    bf16 = mybir.dt.bfloat16
    AF = mybir.ActivationFunctionType

    nc = bacc.Bacc("TRN2", target_bir_lowering=False, debug=False,
                   num_devices=NCORES)

    xT = nc.declare_dram_parameter("xT", [E, T], bf16, isOutput=False)
    wq = nc.declare_dram_parameter("wq", [E, CLOC], bf16, isOutput=False)
    wk = nc.declare_dram_parameter("wk", [E, CLOC], bf16, isOutput=False)
    wv = nc.declare_dram_parameter("wv", [E, CLOC], bf16, isOutput=False)
    wo = nc.declare_dram_parameter("wo", [CLOC, E], bf16, isOutput=False)
    out = nc.declare_dram_parameter("out", [T, E], bf16, isOutput=True)
    DBG = os.environ.get("KERNEL_DBG")
    dbg = (nc.declare_dram_parameter("dbg", [128, 8, 2048], bf16,
                                     isOutput=True)
           if DBG else None)

    KC = E // 128        # 8 contraction chunks for the projections
    NT = T // 128        # 16 T'-tiles (key rows per tile)
    NTC = T // 512       # 4 t-chunks (query columns per chunk)
    NPAIR = HLOC // 2    # 2 head pairs

    # PSUM budget (8 banks of 2KB/partition):
    #   bigps: shared ring of [128,2,512] tiles (scores, projections,
    #          out-proj) -> 3 bufs x 2 banks = 6 banks
    #   pvps:  2 per-head PV accumulators [65,512]     = 2 banks
    with tile.TileContext(nc) as tc:
        with (
            tc.tile_pool(name="persist", bufs=1) as persist,
            tc.tile_pool(name="bigps", bufs=3, space="PSUM") as bigps,
            tc.tile_pool(name="pvps", bufs=1, space="PSUM") as pvps,
            tc.tile_pool(name="ptile", bufs=6) as ptile,
            tc.tile_pool(name="small", bufs=2) as small,
            tc.tile_pool(name="ostg", bufs=3) as ostgp,
            tc.tile_pool(name="dscratch", bufs=4, space="DRAM") as dscratch,
        ):
            # ---- stage inputs in SBUF (all bf16) ----
            xT_sb = persist.tile([128, KC, T], bf16)
            wq_sb = persist.tile([128, KC, CLOC], bf16)
            wk_sb = persist.tile([128, KC, CLOC], bf16)
            wv_sb = persist.tile([128, KC, CLOC], bf16)
            xT_r = xT.ap().rearrange("(a p) t -> p a t", p=128)
            wq_r = wq.ap().rearrange("(a p) c -> p a c", p=128)
            wk_r = wk.ap().rearrange("(a p) c -> p a c", p=128)
            wv_r = wv.ap().rearrange("(a p) c -> p a c", p=128)
            # sync issues wk + the xT t-half each kc's first matmuls need,
            # interleaved and split by partition halves (same 2KB bursts,
            # more queue parallelism); gpsimd issues the rest; the scalar
            # engine issues nothing so exp starts unimpeded
            for kc in range(KC):
                nc.sync.dma_start(out=wk_sb[:, kc, :], in_=wk_r[:, kc, :])
                nc.sync.dma_start(
                    out=xT_sb[0:64, kc, 0:1024], in_=xT_r[0:64, kc, 0:1024]
                )
                nc.sync.dma_start(
                    out=xT_sb[64:128, kc, 0:1024],
                    in_=xT_r[64:128, kc, 0:1024],
                )
                nc.gpsimd.dma_start(out=wq_sb[:, kc, :], in_=wq_r[:, kc, :])
            for kc in range(KC):
                nc.gpsimd.dma_start(out=wv_sb[:, kc, :], in_=wv_r[:, kc, :])
                nc.sync.dma_start(
                    out=xT_sb[0:64, kc, 1024:T], in_=xT_r[0:64, kc, 1024:T]
                )
                nc.sync.dma_start(
                    out=xT_sb[64:128, kc, 1024:T],
                    in_=xT_r[64:128, kc, 1024:T],
                )
            # Wo rows for head pair pp live at partitions 0..127 of plane pp.
            wo_sb = persist.tile([128, HLOC // 2, E], bf16)
            wo_r = wo.ap().rearrange("(pp r) e -> r pp e", r=128)
            for pp in range(HLOC // 2):
                nc.gpsimd.dma_start(out=wo_sb[:, pp, :], in_=wo_r[:, pp, :])

            # ---- projections ----
            # Q^T, K^T: [CLOC, T] with head-local d on partitions
            # (M-group mg holds heads 2mg, 2mg+1).
            qT_sb = persist.tile([128, NPAIR, T], bf16)
            kT_sb = persist.tile([128, NPAIR, T], bf16)
            # V per t-tile: [t(128), tile, head, 65] with the ones column
            # for the softmax denominator at column 64 of each head slot.
            vp_sb = persist.tile([128, NT, HLOC, D + 1], bf16)
            nc.vector.memset(vp_sb[:, :, :, D : D + 1], 1.0)

            def proj_half(w_sb, dst, mg, tc2):
                """One projection group: head pair mg, t-half tc2. The two
                512-col halves evict separately so consumers unblock as soon
                as their half lands."""
                ps = bigps.tile([128, 2, 512], f32, tag="big")
                for half in range(2):
                    tcn = 2 * tc2 + half
                    for kc in range(KC):
                        nc.tensor.matmul(
                            ps[:, half, :],
                            lhsT=w_sb[:, kc, mg * 128 : (mg + 1) * 128],
                            rhs=xT_sb[:, kc, tcn * 512 : (tcn + 1) * 512],
                            start=(kc == 0),
                            stop=(kc == KC - 1),
                        )
                    nc.vector.tensor_copy(
                        out=dst[
                            :, mg, tc2 * 1024 + half * 512 :
                            tc2 * 1024 + (half + 1) * 512
                        ],
                        in_=ps[:, half, :],
                    )

            def v_proj_pair(tt0):
                # V tiles tt0, tt0+1 in natural [t, c] layout via x^T as the
                # stationary side; both share one ring slot and one eviction
                # copy so the jit projection doesn't thrash the scores ring
                ps = bigps.tile([128, 2, 512], f32, tag="big")
                for half in range(2):
                    tt = tt0 + half
                    for kc in range(KC):
                        nc.tensor.matmul(
                            ps[:, half, 0:CLOC],
                            lhsT=xT_sb[:, kc, tt * 128 : (tt + 1) * 128],
                            rhs=wv_sb[:, kc, :],
                            start=(kc == 0),
                            stop=(kc == KC - 1),
                        )
                nc.vector.tensor_copy(
                    out=vp_sb[:, tt0 : tt0 + 2, :, 0:D],
                    in_=ps[:, :, 0:CLOC].rearrange(
                        "p a (h d) -> p a h d", h=HLOC
                    ),
                )

            # headsN[c_lo, pair, t]: plane `pair` holds heads 2p (partitions
            # 0..63) and 2p+1 (64..127) — ready as out-proj stationary tiles.
            headsN = persist.tile([128, NPAIR, T], bf16)

            # per-chunk PV accumulators, live across the g-split
            pv_cur = [None, None]

            def attention(pair, tcn, g0, g1, jit_vproj=False):
                """Scores+exp+PV for T'-tile pairs g0..g1-1 of chunk
                (pair, tcn); caller finishes the chunk with attn_norm."""
                h0, h1 = 2 * pair, 2 * pair + 1
                tsl = slice(tcn * 512, (tcn + 1) * 512)
                if g0 == 0:
                    pv_cur[0] = pvps.tile([65, 512], f32, name="pv0", tag="pv0")
                    pv_cur[1] = pvps.tile([65, 512], f32, name="pv1", tag="pv1")
                pv0, pv1 = pv_cur
                for g in range(g0, g1):
                    st = (g == 0)
                    sp = (g == NT // 2 - 1)
                    pss, pts = [], []
                    last_s = None
                    for i in range(2):
                        tt = 2 * g + i
                        ksl = slice(tt * 128, (tt + 1) * 128)
                        ps_s = bigps.tile([128, 2, 512], f32, tag="big")
                        pss.append(ps_s)
                        # S^T for both heads (row-packed on the PE)
                        nc.tensor.matmul(
                            ps_s[:, 0, :],
                            lhsT=kT_sb[0:64, pair, ksl],
                            rhs=qT_sb[0:64, pair, tsl],
                            tile_position=(0, 0),
                        )
                        last_s = nc.tensor.matmul(
                            ps_s[:, 1, :],
                            lhsT=kT_sb[64:128, pair, ksl],
                            rhs=qT_sb[64:128, pair, tsl],
                            tile_position=(64, 0),
                        )
                    for i in range(2):
                        pt = ptile.tile([128, 2, 512], bf16, tag="pt")
                        pts.append(pt)
                        nc.scalar.activation(
                            out=pt[:], in_=pss[i][:], func=AF.Exp,
                            scale=0.125,
                        )
                    if jit_vproj:
                        # first consumer of these V tiles: project them
                        # now so the PE fills exp-wait slack
                        v_proj_pair(2 * g)
                    # same-bank accumulations back-to-back: bank
                    # switches between matmuls break the PE's drain-under-
                    # fill pipelining, so do both of pv0's before pv1's
                    for hh, pv in ((0, pv0), (1, pv1)):
                        for i in range(2):
                            tt = 2 * g + i
                            mm = nc.tensor.matmul(
                                pv[:, :],
                                lhsT=vp_sb[:, tt, 2 * pair + hh, :],
                                rhs=pts[i][:, hh, :],
                                start=(st and i == 0), stop=(sp and i == 1),
                            )
                            if hh == 0 and i == 0:
                                add_dep_helper(
                                    _inst(mm), _inst(last_s),
                                    reason="batch scores before pv",
                                )

            def attn_norm(pair, tcn, fine=False):
                """Finish chunk (pair, tcn): evacuate the two per-head PV
                accumulators, build the broadcast reciprocal denominator,
                normalize into headsN. h1's block is DMA-shifted to
                partitions 64..127. `fine` splits the multiplies per
                128-col out-proj tile to shorten the end-of-kernel tail."""
                tsl = slice(tcn * 512, (tcn + 1) * 512)
                pv0, pv1 = pv_cur
                stgA = small.tile([65, 512], f32, tag="stgA")
                nc.vector.tensor_copy(out=stgA[:], in_=pv0[:])
                stgB = small.tile([65, 512], f32, tag="stgB")
                nc.vector.tensor_copy(out=stgB[:], in_=pv1[:])
                # bounce the raw denominator rows through DRAM to
                # broadcast along partitions (stride-0 SBUF DMA sources and
                # gpsimd partition_broadcast from a base-64 row both fail).
                # NB: reciprocal_approx_fast is a custom-DVE op; running it
                # in-place on a [1,512] single-partition row corrupts
                # unrelated SBUF -- keep it out-of-place on the broadcast.
                dscA = dscratch.tile([1, 512], f32, tag="dscA")
                nc.sync.dma_start(out=dscA[:], in_=stgA[64:65, :])
                dscB = dscratch.tile([1, 512], f32, tag="dscB")
                nc.gpsimd.dma_start(out=dscB[:], in_=stgB[64:65, :])
                den0 = small.tile([64, 512], f32, tag="den0")
                nc.sync.dma_start(
                    out=den0[:], in_=dscA[:].to_broadcast([64, 512])
                )
                den1 = small.tile([64, 512], f32, tag="den1")
                nc.sync.dma_start(
                    out=den1[:], in_=dscB[:].to_broadcast([64, 512])
                )
                rec0 = small.tile([64, 512], f32, tag="rec0")
                nc.vector.reciprocal_approx_fast(out=rec0[:], in_=den0[:])
                rec1 = small.tile([64, 512], f32, tag="rec1")
                nc.vector.reciprocal_approx_fast(out=rec1[:], in_=den1[:])
                cols = 4 if fine else 1
                w = 512 // cols
                for j in range(cols):
                    jsl = slice(j * w, (j + 1) * w)
                    hsl = slice(tcn * 512 + j * w, tcn * 512 + (j + 1) * w)
                    nc.vector.tensor_mul(
                        out=headsN[0:64, pair, hsl],
                        in0=stgA[0:64, jsl], in1=rec0[:, jsl],
                    )
                    tmpB = small.tile([64, 512], bf16, tag="tmpB")
                    nc.vector.tensor_mul(
                        out=tmpB[:, jsl], in0=stgB[0:64, jsl],
                        in1=rec1[:, jsl],
                    )
                    nc.sync.dma_start(
                        out=headsN[64:128, pair, hsl], in_=tmpB[:, jsl]
                    )

            def out_tile(tt):
                # partial[t-tile] = headsN^T @ Wo_loc ; both e-chunks share
                # one PSUM tile; staging copy on gpsimd keeps vector free
                ksl = slice(tt * 128, (tt + 1) * 128)
                ps_o = bigps.tile([128, 2, 512], f32, tag="big")
                for ec in range(2):
                    esl = slice(ec * 512, (ec + 1) * 512)
                    for pp in range(NPAIR):
                        nc.tensor.matmul(
                            ps_o[:, ec, :],
                            lhsT=headsN[:, pp, ksl],
                            rhs=wo_sb[:, pp, esl],
                            start=(pp == 0),
                            stop=(pp == NPAIR - 1),
                        )
                stg = ostgp.tile([128, 2, 512], bf16, tag="ostg")
                nc.vector.tensor_copy(out=stg[:], in_=ps_o[:])
                nc.sync.dma_start(
                    out=out.ap()[ksl, :],
                    in_=stg[:].rearrange("p a b -> p (a b)"),
                )

            # Emission order IS program order (Tile tracks deps in trace
            # order) and doubles as scheduler priority. Only what the next
            # attention piece needs is projected first (scores for tile
            # pairs g0..3 want just the first K t-half and the current Q
            # chunk) so the scalar engine starts exp as early as possible;
            # V tiles are projected just-in-time inside pair-0 tcn-0;
            # out-proj tiles stream after each pair-1 chunk.
            for pair in range(NPAIR):
                proj_half(wk_sb, kT_sb, pair, 0)
                proj_half(wq_sb, qT_sb, pair, 0)
                attention(pair, 0, 0, 4, jit_vproj=(pair == 0))
                proj_half(wk_sb, kT_sb, pair, 1)
                attention(pair, 0, 4, 8, jit_vproj=(pair == 0))
                attn_norm(pair, 0)
                attention(pair, 1, 0, 8)
                attn_norm(pair, 1)
                if pair == 1:
                    # one-chunk lag: when the PE reaches these, the norm
                    # chain they depend on completed a whole chunk ago
                    for tt in range(0, 4):
                        out_tile(tt)
                proj_half(wq_sb, qT_sb, pair, 1)
                attention(pair, 2, 0, 8)
                attn_norm(pair, 2)
                if pair == 1:
                    for tt in range(4, 8):
                        out_tile(tt)
                attention(pair, 3, 0, 4)
                if pair == 1:
                    for tt in range(8, 12):
                        out_tile(tt)
                attention(pair, 3, 4, 8)
                attn_norm(pair, 3, fine=(pair == 1))
                if pair == 1:
                    for tt in range(12, 16):
                        out_tile(tt)
            if dbg is not None:
                if DBG == "xt":
                    for kc in range(KC):
                        nc.sync.dma_start(
                            out=dbg.ap()[:, kc, :], in_=xT_sb[:, kc, :]
                        )
                elif DBG == "qt":
                    nc.sync.dma_start(out=dbg.ap()[:, 0, :], in_=qT_sb[:, 0, :])
                    nc.sync.dma_start(out=dbg.ap()[:, 1, :], in_=qT_sb[:, 1, :])
                    nc.sync.dma_start(out=dbg.ap()[:, 2, :], in_=kT_sb[:, 0, :])
                    nc.sync.dma_start(out=dbg.ap()[:, 3, :], in_=kT_sb[:, 1, :])
                elif DBG == "heads":
                    nc.sync.dma_start(out=dbg.ap()[:, 0, :], in_=headsN[:, 0, :])
                    nc.sync.dma_start(out=dbg.ap()[:, 1, :], in_=headsN[:, 1, :])

    nc.compile()
    return nc


def _get_program():
    global _cached_nc
    if _cached_nc is None:
        _cached_nc = _build_program()
    return _cached_nc


def kernel(x, Wq, Wk, Wv, Wo):
    global LAST_EXEC_NS
    _ensure_ntff_hook()
    from concourse.bass_utils import run_bass_kernel_spmd

    nc = _get_program()
    bf16 = ml_dtypes.bfloat16

    x = np.asarray(x, dtype=np.float32)
    in_maps = []
    for c in range(NCORES):
        b = c // TPC
        hs = (c % TPC) * HLOC
        xT_c = np.ascontiguousarray(x[b].T).astype(bf16)
        # [HLOC, E, D] -> [E, HLOC*D]
        wq_c = np.ascontiguousarray(
            np.asarray(Wq)[hs : hs + HLOC].transpose(1, 0, 2).reshape(E, CLOC)
        ).astype(bf16)
        wk_c = np.ascontiguousarray(
            np.asarray(Wk)[hs : hs + HLOC].transpose(1, 0, 2).reshape(E, CLOC)
        ).astype(bf16)
        wv_c = np.ascontiguousarray(
            np.asarray(Wv)[hs : hs + HLOC].transpose(1, 0, 2).reshape(E, CLOC)
        ).astype(bf16)
        wo_c = np.ascontiguousarray(
            np.asarray(Wo)[hs * D : (hs + HLOC) * D, :]
        ).astype(bf16)
        in_maps.append(
            {"xT": xT_c, "wq": wq_c, "wk": wk_c, "wv": wv_c, "wo": wo_c}
        )

    trace = bool(os.environ.get("KERNEL_TRACE"))
    res = run_bass_kernel_spmd(nc, in_maps, list(range(NCORES)), trace=trace)
    LAST_EXEC_NS = res.exec_time_ns

    out = np.empty((B, T, E), dtype=np.float32)
    for b in range(B):
        acc = x[b].copy()
        for g in range(TPC):
            acc += np.asarray(res.results[b * TPC + g]["out"],
                              dtype=np.float32)
        out[b] = acc
    return out
